# revision 13
# baseline (speedup 1.0000x reference)
"""Causal self-attention on 8 Trainium2 NeuronCores.

Problem: B=2, T=2048, E=1024, H=16 heads (D=64), fp32.
  qkv = x @ W_qkv + b_qkv ; causal softmax attention ; y @ W_out + b_out

Sharding: core c handles batch b = c//4 and head group g = c%4 (4 heads,
256 of the 1024 hidden dims).  QKV + attention are computed fully
locally per core (tensor-parallel on heads, data-parallel on batch).
The output projection is Megatron ROW-split: each core multiplies its
own y rows [256, q] by W_out[own 256 rows, all 1024 cols], producing a
full-width partial out.T [1024, 512] per q-tile (bf16), and a per-tile
ReduceScatter(add) within the batch group sums the partials; a final
bias+fp32-cast pass lands each core's 256-row shard in the output.
Unlike an AllGather formulation this puts no collective on the critical
path of any matmul; only the last tile's ReduceScatter trails compute.

x / W_qkv / W_out are pre-cast to bf16 on the host (numerically
identical to the on-device cast the kernel would otherwise do, but
halves input DMA bytes and frees the DVE).  PSUM accumulation is fp32
throughout.  Attention uses the transposed-scores layout: S.T[k, q]
tiles so the softmax denominator comes from an appended ones-column in
the V stationary operand and exp() runs on the Scalar engine straight
out of PSUM.  Causal masking is an additive -1e9 on the diagonal
k-chunks.  Per q-tile the heads are software-pipelined
(S0 S1 [outproj j-1] A0 S2 A1 S3 A2 A3) so Scalar-engine exp of head
h+1 overlaps the AV matmuls of head h — exp is the binding engine of
the attention phase — while keeping the PE stream dense enough to hold
the high p-state (the PE ramps 1.2 -> 2.4 GHz only after ~3us of
continuous execution).  The softmax normalize (single-op DVE
reciprocal approx, ~18 bits -> partition broadcast -> multiply) is
emitted per head, off the PE stream.  A tiny warmup AllGather absorbs
cross-core launch skew, triggered only after every input-DMA issue so
it cannot stall the x stream.
"""

import numpy as np

import concourse.bass as bass
import concourse.mybir as mybir
import concourse.tile as tile
from concourse import bacc
from concourse.bass_utils import run_bass_kernel_spmd

F32 = mybir.dt.float32
BF16 = mybir.dt.bfloat16
AF = mybir.ActivationFunctionType
OP = mybir.AluOpType

B, T, E, H = 2, 2048, 1024, 16
D = E // H            # 64 head dim
HL = 4                # heads per core
DL = HL * D           # 256 local hidden dims per core
NE = E // 128         # 8 contraction chunks
NT = T // 128         # 16 t-chunks
NJ = T // 512         # 4 q-tiles
SCALE = 1.0 / float(np.sqrt(D))
NEG = -1.0e9
GROUPS = [[0, 1, 2, 3], [4, 5, 6, 7]]

_CACHE = {}


def build_bass(dt=BF16):
    nc = bacc.Bacc("TRN2", target_bir_lowering=False, debug=False, num_devices=8)

    xT = nc.dram_tensor("xT", [E, T], dt, kind="ExternalInput")
    wqkv = nc.dram_tensor("wqkv", [E, 3 * DL], dt, kind="ExternalInput")
    bqkv = nc.dram_tensor("bqkv", [6, 128], F32, kind="ExternalInput")
    wout = nc.dram_tensor("wout", [DL, E], dt, kind="ExternalInput")
    bout4 = nc.dram_tensor("bout4", [NE, 128], F32, kind="ExternalInput")
    ones2 = nc.dram_tensor("ones2", [128, NT * HL], dt, kind="ExternalInput")
    ident = nc.dram_tensor("ident", [128, 128], dt, kind="ExternalInput")
    out_shard = nc.dram_tensor("out_shard", [NJ, DL, 512], dt,
                               kind="ExternalOutput")

    # partial out.T per q-tile, ReduceScatter'd within the batch group.
    # One tensor PER TILE: dram dependency tracking is per-tensor, so a
    # shared tensor would chain tile j's partial writes behind the
    # in-flight ReduceScatter of tile j-1.
    partials = [nc.dram_tensor(f"partials{j}", [E, 512], dt)
                for j in range(NJ)]
    rs_out = [nc.dram_tensor(f"rs_out{j}", [DL, 512], dt)
              for j in range(NJ)]
    warm_in = nc.dram_tensor("warm_in", [1, 512], F32)
    warm_out = nc.dram_tensor("warm_out", [4, 512], F32)

    with tile.TileContext(nc) as tc:
        with tc.tile_pool(name="const", bufs=1) as constp, \
             tc.tile_pool(name="qkvs", bufs=1) as qkvp:
            bq_s = constp.tile([128, 6], F32)
            bo_s = constp.tile([128, NE], F32)
            tri = constp.tile([128, 128], F32)
            warm_s = constp.tile([1, 512], F32)
            id_s = constp.tile([128, 128], dt)

            QT_s = qkvp.tile([128, 2, T], dt)
            KT_s = qkvp.tile([128, 2, T], dt)
            V_s = qkvp.tile([128, NT, HL, D + 1], dt)
            VT_s = qkvp.tile([128, 2, T], dt)
            wo_s = qkvp.tile([128, 2, E], dt)

            # ---------------- phase 1: QKV projections ----------------
            with tc.tile_pool(name="in1", bufs=1) as in1, \
                 tc.tile_pool(name="ps1", bufs=8, space="PSUM") as ps1:
                xT_r = xT.ap().rearrange("(c p) t -> p c t", p=128)
                wq_r = wqkv.ap().rearrange("(c p) m -> p c m", p=128)
                wo_r = wout.ap().rearrange("(c p) m -> p c m", p=128)

                x_s = in1.tile([128, NE, T], dt, tag="xb")
                w_s = in1.tile([128, NE, 3 * DL], dt, tag="wb")

                # x stream alone on the gpsimd queue; w + small constants
                # on the sync queue
                nc.sync.dma_start(out=w_s[:, 0, :], in_=wq_r[:, 0, :])
                nc.gpsimd.dma_start(out=x_s[:, 0, :], in_=xT_r[:, 0, :])
                nc.sync.dma_start(
                    out=bq_s[:], in_=bqkv.ap().rearrange("m p -> p m"))
                nc.sync.dma_start(
                    out=bo_s[:], in_=bout4.ap().rearrange("m p -> p m"))
                nc.sync.dma_start(
                    out=V_s[:, :, :, D],
                    in_=ones2.ap().rearrange("p (a b) -> p a b", a=NT))
                nc.gpsimd.memset(warm_s[:], 0.0)
                nc.sync.dma_start(out=warm_in.ap(), in_=warm_s[:])
                nc.sync.dma_start(out=id_s[:], in_=ident.ap())
                for ec in range(1, NE):
                    nc.sync.dma_start(out=w_s[:, ec, :], in_=wq_r[:, ec, :])
                    if ec < 4:
                        nc.gpsimd.dma_start(out=x_s[:, ec, :],
                                            in_=xT_r[:, ec, :])
                for ec in range(4, NE):
                    nc.sync.dma_start(out=x_s[:, ec, :], in_=xT_r[:, ec, :])
                for c2 in range(2):
                    nc.sync.dma_start(out=wo_s[:, c2, :], in_=wo_r[:, c2, :])
                # warmup collective: rendezvous the group early so launch
                # skew is absorbed here, not in the first ReduceScatter.
                # Triggered after every input DMA issue — the trigger
                # blocks the gpsimd stream until warm_in lands.
                nc.gpsimd.collective_compute(
                    "AllGather", OP.bypass, replica_groups=GROUPS,
                    ins=[warm_in.ap()], outs=[warm_out.ap()])
                nc.gpsimd.memset(tri[:], 0.0)
                nc.gpsimd.affine_select(
                    out=tri[:], in_=tri[:], compare_op=OP.is_ge,
                    fill=NEG, base=0, pattern=[[1, 128]],
                    channel_multiplier=-1)

                # Q.T / K.T : [cols 256, T] each, cols on partitions.
                # Two m-chunks at a time (8 PSUM banks) so the PE has 2x
                # work per x-chunk while the input DMA is still streaming.
                for mp in range(3):
                    pss = [ps1.tile([128, 512], F32, tag="psQ",
                                    name=f"psQ{mp}_{_i}") for _i in range(8)]
                    for ec in range(NE):
                        for mi in range(2):
                            m = 2 * mp + mi
                            for nt in range(NJ):
                                nc.tensor.matmul(
                                    pss[4 * mi + nt][:],
                                    w_s[:, ec, m * 128:(m + 1) * 128],
                                    x_s[:, ec, nt * 512:(nt + 1) * 512],
                                    start=(ec == 0), stop=(ec == NE - 1))
                    for mi in range(2):
                        m = 2 * mp + mi
                        dest = (QT_s, QT_s, KT_s, KT_s, VT_s, VT_s)[m]
                        bias = bq_s[:, m:m + 1] if m < 4 else 0.0
                        mm = m % 2
                        for nt in range(NJ):
                            nc.scalar.activation(
                                dest[:, mm, nt * 512:(nt + 1) * 512],
                                pss[4 * mi + nt][:],
                                AF.Identity, bias=bias, scale=1.0)


            # V.T -> V natural [t, vcol] via PE transpose (bias folded
            # in later via attention row-sums); own PSUM pool, after ps1
            # is released
            with tc.tile_pool(name="psT", bufs=4, space="PSUM") as psTp:
                for mt in range(NT):
                    for c in range(2):
                        psT = psTp.tile([128, 128], dt, tag="psT",
                                        name=f"psT{mt}_{c}")
                        nc.tensor.transpose(
                            psT[:], VT_s[:, c, mt * 128:(mt + 1) * 128],
                            id_s[:])
                        nc.vector.tensor_copy(
                            V_s[:, mt, 2 * c:2 * c + 2, 0:D],
                            psT[:].rearrange("p (a b) -> p a b", a=2))

            # ------- phase 2: attention + row-split out proj + RS -------
            with tc.tile_pool(name="attn", bufs=2) as attnp, \
                 tc.tile_pool(name="exps", bufs=4) as expsp, \
                 tc.tile_pool(name="out3", bufs=3) as out3, \
                 tc.tile_pool(name="psS", bufs=2, space="PSUM") as psSp, \
                 tc.tile_pool(name="psO", bufs=2, space="PSUM") as psOp, \
                 tc.tile_pool(name="ps3", bufs=2, space="PSUM") as ps3:
                OTns = {}

                def emit_outproj(jj):
                    OTp = OTns.pop(jj)
                    last = jj == NJ - 1
                    for m in range(NE):
                        ps = ps3.tile([128, 512], F32, tag="psP",
                                      name=f"psP{jj}_{m}")
                        for c2 in range(2):
                            nc.tensor.matmul(
                                ps[:],
                                wo_s[:, c2, m * 128:(m + 1) * 128],
                                OTp[:, c2, :],
                                start=(c2 == 0), stop=(c2 == 1))
                        ot = out3.tile([128, 512], dt, tag="ot",
                                       name=f"ot{jj}_{m}")
                        # + b_out/4: summed across the 4-core group by
                        # the ReduceScatter, this lands the exact bias
                        if last:
                            # the DVE is busy with the last tile's softmax
                            # normalize here; Scalar has finished all exp
                            nc.scalar.add(ot[:], ps[:], bo_s[:, m:m + 1])
                        else:
                            nc.vector.tensor_scalar_add(
                                ot[:], ps[:], bo_s[:, m:m + 1])
                        nc.sync.dma_start(
                            out=partials[jj][m * 128:(m + 1) * 128, :],
                            in_=ot[:])
                    nc.gpsimd.collective_compute(
                        "ReduceScatter", OP.add, replica_groups=GROUPS,
                        ins=[partials[jj].ap()], outs=[rs_out[jj].ap()])

                for j in range(NJ):
                    OTn = attnp.tile([128, 2, 512], dt, tag="otn",
                                     name=f"otn{j}")
                    OTns[j] = OTn
                    nkc = 4 * j + 4
                    ex_tiles = [None] * HL

                    def emit_scores(h, j=j, nkc=nkc, ex_tiles=ex_tiles):
                        b64 = 64 * (h % 2)
                        hh = h // 2
                        expS = expsp.tile([128, NT, 512], dt, tag="expS",
                                          name=f"exp{j}_{h}")
                        ex_tiles[h] = expS
                        for kp in range(nkc // 2):
                            ps = psSp.tile([128, 2, 512], F32, tag="psS")
                            offs = []
                            for half in range(2):
                                kc = 2 * kp + half
                                # columns q' < off are fully masked: the
                                # matmul, exp and AV all skip them
                                off = max(0, 128 * kc - 512 * j)
                                offs.append(off)
                                nc.tensor.matmul(
                                    ps[:, half, off:512],
                                    KT_s[b64:b64 + 64, hh,
                                         kc * 128:(kc + 1) * 128],
                                    QT_s[b64:b64 + 64, hh,
                                         j * 512 + off:(j + 1) * 512],
                                    start=True, stop=True)
                                if 128 * kc >= 512 * j:
                                    # diagonal block: additive triangle
                                    nc.vector.tensor_tensor(
                                        out=ps[:, half, off:off + 128],
                                        in0=ps[:, half, off:off + 128],
                                        in1=tri[:], op=OP.add)
                            if offs == [0, 0]:
                                nc.scalar.activation(
                                    expS[:, 2 * kp:2 * kp + 2, :], ps[:],
                                    AF.Exp, scale=SCALE)
                            else:
                                for half in range(2):
                                    kc = 2 * kp + half
                                    nc.scalar.activation(
                                        expS[:, kc, offs[half]:512],
                                        ps[:, half, offs[half]:512],
                                        AF.Exp, scale=SCALE)

                    def emit_av(h, j=j, nkc=nkc, ex_tiles=ex_tiles, OTn=OTn):
                        b64 = 64 * (h % 2)
                        hh = h // 2
                        po = psOp.tile([D + 1, 512], F32, tag="psO")
                        for kc in range(nkc):
                            off = max(0, 128 * kc - 512 * j)
                            nc.tensor.matmul(
                                po[:, off:512], V_s[:, kc, h, :],
                                ex_tiles[h][:, kc, off:512],
                                start=(kc == 0), stop=(kc == nkc - 1))
                        # normalize head h off the PE stream
                        dn = attnp.tile([1, 512], F32, tag="dn")
                        rr = attnp.tile([1, 512], F32, tag="rr")
                        rba = attnp.tile([64, 512], F32, tag="rba")
                        nc.vector.tensor_copy(dn[:], po[D:D + 1, :])
                        nc.vector.reciprocal_approx_fast(rr[:], dn[:])
                        nc.gpsimd.partition_broadcast(rba[:], rr[:])
                        dst = OTn[b64:b64 + 64, hh, :]
                        nc.vector.tensor_tensor(
                            out=dst, in0=po[0:D, :], in1=rba[:], op=OP.mult)
                        # + b_qkv V-slice (attention rows sum to 1)
                        nc.vector.tensor_scalar_add(
                            dst, dst, bq_s[b64:b64 + 64, 4 + hh:5 + hh])

                    # head-pipelined schedule: exp(h+1) overlaps AV(h)
                    emit_scores(0)
                    emit_scores(1)
                    if j > 0:
                        emit_outproj(j - 1)
                    emit_av(0)
                    emit_scores(2)
                    emit_av(1)
                    emit_scores(3)
                    emit_av(2)
                    emit_av(3)
                emit_outproj(NJ - 1)
                # land the scattered shards; emitted last so no engine
                # stream ever blocks waiting for an in-flight collective
                for j in range(NJ):
                    nc.scalar.dma_start(out=out_shard[j], in_=rs_out[j].ap())
    nc.compile()
    return nc


def _get_nc(dt=BF16):
    key = ("nc", dt)
    if key not in _CACHE:
        _CACHE[key] = build_bass(dt)
    return _CACHE[key]


def kernel(x, W_qkv, b_qkv, W_out, b_out, dt=BF16, **run_kwargs):
    import ml_dtypes
    bf16 = ml_dtypes.bfloat16
    x = np.asarray(x, np.float32)
    W_qkv = np.asarray(W_qkv, np.float32)
    b_qkv = np.asarray(b_qkv, np.float32)
    W_out = np.asarray(W_out, np.float32)
    b_out = np.asarray(b_out, np.float32)

    ones2 = np.ones((128, NT * HL), bf16)
    ident = np.eye(128, dtype=bf16)
    in_maps = []
    for c in range(8):
        b, g = divmod(c, 4)
        cols = slice(g * DL, (g + 1) * DL)
        wq = W_qkv[:, 0 * E:1 * E][:, cols]
        wk = W_qkv[:, 1 * E:2 * E][:, cols]
        wv = W_qkv[:, 2 * E:3 * E][:, cols]
        bq = b_qkv[0 * E:1 * E][cols]
        bk = b_qkv[1 * E:2 * E][cols]
        bv = b_qkv[2 * E:3 * E][cols]
        in_maps.append({
            "xT": np.ascontiguousarray(x[b].T).astype(bf16),
            "wqkv": np.concatenate([wq, wk, wv], axis=1).astype(bf16),
            "bqkv": np.concatenate([bq, bk, bv]).reshape(6, 128),
            "wout": np.ascontiguousarray(W_out[cols, :]).astype(bf16),
            "bout4": (b_out / 4.0).reshape(NE, 128),
            "ones2": ones2,
            "ident": ident,
        })

    res = run_bass_kernel_spmd(_get_nc(dt), in_maps, list(range(8)), **run_kwargs)
    _CACHE["last_results"] = res

    out = np.empty((B, T, E), np.float32)
    for c in range(8):
        b, g = divmod(c, 4)
        shard = res.results[c]["out_shard"]          # [NJ, DL, 512] bf16
        full = shard.transpose(1, 0, 2).reshape(DL, T).astype(np.float32)
        out[b][:, g * DL:(g + 1) * DL] = full.T
    return out


# revision 14
# speedup vs baseline: 1.1773x; 1.1773x over previous
"""Causal self-attention on 8 Trainium2 NeuronCores.

Problem: B=2, T=2048, E=1024, H=16 heads (D=64), fp32.
  qkv = x @ W_qkv + b_qkv ; causal softmax attention ; y @ W_out + b_out

Sharding: core c handles batch b = c//4 and head group g = c%4 (4 heads,
256 of the 1024 hidden dims).  QKV + attention are computed fully
locally per core (tensor-parallel on heads, data-parallel on batch).
The output projection is Megatron ROW-split: each core multiplies its
own y rows [256, q] by W_out[own 256 rows, all 1024 cols], producing a
full-width partial out.T [1024, 512] per q-tile (bf16), and a per-tile
ReduceScatter(add) within the batch group sums the partials; a final
bias+fp32-cast pass lands each core's 256-row shard in the output.
Unlike an AllGather formulation this puts no collective on the critical
path of any matmul; only the last tile's ReduceScatter trails compute.

x / W_qkv / W_out are pre-cast to bf16 on the host (numerically
identical to the on-device cast the kernel would otherwise do, but
halves input DMA bytes and frees the DVE).  PSUM accumulation is fp32
throughout.  Attention uses the transposed-scores layout: S.T[k, q]
tiles so the softmax denominator comes from an appended ones-column in
the V stationary operand and exp() runs on the Scalar engine straight
out of PSUM.  Causal masking is an additive -1e9 on the diagonal
k-chunks.  Per q-tile the heads are software-pipelined
(S0 S1 [outproj j-1] A0 S2 A1 S3 A2 A3) so Scalar-engine exp of head
h+1 overlaps the AV matmuls of head h — exp is the binding engine of
the attention phase — while keeping the PE stream dense enough to hold
the high p-state (the PE ramps 1.2 -> 2.4 GHz only after ~3us of
continuous execution).  The softmax normalize (single-op DVE
reciprocal approx, ~18 bits -> partition broadcast -> multiply) is
emitted per head, off the PE stream.  A tiny warmup AllGather absorbs
cross-core launch skew, triggered only after every input-DMA issue so
it cannot stall the x stream.
"""

import numpy as np

import concourse.bass as bass
import concourse.mybir as mybir
import concourse.tile as tile
from concourse import bacc
from concourse.bass_utils import run_bass_kernel_spmd

F32 = mybir.dt.float32
BF16 = mybir.dt.bfloat16
AF = mybir.ActivationFunctionType
OP = mybir.AluOpType

B, T, E, H = 2, 2048, 1024, 16
D = E // H            # 64 head dim
HL = 4                # heads per core
DL = HL * D           # 256 local hidden dims per core
NE = E // 128         # 8 contraction chunks
NT = T // 128         # 16 t-chunks
NJ = T // 512         # 4 q-tiles
SCALE = 1.0 / float(np.sqrt(D))
NEG = -1.0e9
GROUPS = [[0, 1, 2, 3], [4, 5, 6, 7]]

_CACHE = {}


def build_bass(dt=BF16):
    nc = bacc.Bacc("TRN2", target_bir_lowering=False, debug=False, num_devices=8)

    xT = nc.dram_tensor("xT", [E, T], dt, kind="ExternalInput")
    wqkv = nc.dram_tensor("wqkv", [E, 3 * DL], dt, kind="ExternalInput")
    bqkv = nc.dram_tensor("bqkv", [6, 128], F32, kind="ExternalInput")
    wout = nc.dram_tensor("wout", [E, DL], dt, kind="ExternalInput")
    bout = nc.dram_tensor("bout", [2, 128], F32, kind="ExternalInput")
    ones2 = nc.dram_tensor("ones2", [128, NT * HL], dt, kind="ExternalInput")
    ident = nc.dram_tensor("ident", [128, 128], dt, kind="ExternalInput")
    out_shard = nc.dram_tensor("out_shard", [NJ, DL, 512], dt,
                               kind="ExternalOutput")

    # per-tile y.T staging for the pipelined AllGather.  One tensor PER
    # TILE: dram dependency tracking is per-tensor, so a shared tensor
    # would chain tile j's writes behind the in-flight gather of j-1.
    ylocal = [nc.dram_tensor(f"ylocal{j}", [DL, 512], dt)
              for j in range(NJ)]
    ytfull = [nc.dram_tensor(f"ytfull{j}", [E, 512], dt)
              for j in range(NJ)]
    warm_in = nc.dram_tensor("warm_in", [1, 512], F32)
    warm_out = nc.dram_tensor("warm_out", [4, 512], F32)

    with tile.TileContext(nc) as tc:
        with tc.tile_pool(name="const", bufs=1) as constp, \
             tc.tile_pool(name="qkvs", bufs=1) as qkvp:
            bq_s = constp.tile([128, 6], F32)
            bo_s = constp.tile([128, 2], F32)
            tri = constp.tile([128, 128], F32)
            warm_s = constp.tile([1, 512], F32)
            id_s = constp.tile([128, 128], dt)

            QT_s = qkvp.tile([128, 2, T], dt)
            KT_s = qkvp.tile([128, 2, T], dt)
            V_s = qkvp.tile([128, NT, HL, D + 1], dt)
            VT_s = qkvp.tile([128, 2, T], dt)
            wo_s = qkvp.tile([128, NE, DL], dt)

            # ---------------- phase 1: QKV projections ----------------
            with tc.tile_pool(name="in1", bufs=1) as in1, \
                 tc.tile_pool(name="ps1", bufs=8, space="PSUM") as ps1:
                xT_r = xT.ap().rearrange("(c p) t -> p c t", p=128)
                wq_r = wqkv.ap().rearrange("(c p) m -> p c m", p=128)
                wo_r = wout.ap().rearrange("(c p) m -> p c m", p=128)

                x_s = in1.tile([128, NE, T], dt, tag="xb")
                w_s = in1.tile([128, NE, 3 * DL], dt, tag="wb")

                # x stream alone on the gpsimd queue; w + small constants
                # on the sync queue
                nc.sync.dma_start(out=w_s[:, 0, :], in_=wq_r[:, 0, :])
                nc.gpsimd.dma_start(out=x_s[:, 0, :], in_=xT_r[:, 0, :])
                nc.sync.dma_start(
                    out=bq_s[:], in_=bqkv.ap().rearrange("m p -> p m"))
                nc.sync.dma_start(
                    out=bo_s[:], in_=bout.ap().rearrange("m p -> p m"))
                nc.sync.dma_start(
                    out=V_s[:, :, :, D],
                    in_=ones2.ap().rearrange("p (a b) -> p a b", a=NT))
                nc.gpsimd.memset(warm_s[:], 0.0)
                nc.sync.dma_start(out=warm_in.ap(), in_=warm_s[:])
                nc.sync.dma_start(out=id_s[:], in_=ident.ap())
                for ec in range(1, NE):
                    nc.sync.dma_start(out=w_s[:, ec, :], in_=wq_r[:, ec, :])
                    if ec < 4:
                        nc.gpsimd.dma_start(out=x_s[:, ec, :],
                                            in_=xT_r[:, ec, :])
                for ec in range(4, NE):
                    nc.sync.dma_start(out=x_s[:, ec, :], in_=xT_r[:, ec, :])
                for ec in range(NE):
                    nc.sync.dma_start(out=wo_s[:, ec, :], in_=wo_r[:, ec, :])
                # warmup collective: rendezvous the group early so launch
                # skew is absorbed here, not in the first ReduceScatter.
                # Triggered after every input DMA issue — the trigger
                # blocks the gpsimd stream until warm_in lands.
                nc.gpsimd.collective_compute(
                    "AllGather", OP.bypass, replica_groups=GROUPS,
                    ins=[warm_in.ap()], outs=[warm_out.ap()])
                nc.gpsimd.memset(tri[:], 0.0)
                nc.gpsimd.affine_select(
                    out=tri[:], in_=tri[:], compare_op=OP.is_ge,
                    fill=NEG, base=0, pattern=[[1, 128]],
                    channel_multiplier=-1)

                # Q.T / K.T : [cols 256, T] each, cols on partitions.
                # Two m-chunks at a time (8 PSUM banks) so the PE has 2x
                # work per x-chunk while the input DMA is still streaming.
                for mp in range(3):
                    pss = [ps1.tile([128, 512], F32, tag="psQ",
                                    name=f"psQ{mp}_{_i}") for _i in range(8)]
                    for ec in range(NE):
                        for mi in range(2):
                            m = 2 * mp + mi
                            for nt in range(NJ):
                                nc.tensor.matmul(
                                    pss[4 * mi + nt][:],
                                    w_s[:, ec, m * 128:(m + 1) * 128],
                                    x_s[:, ec, nt * 512:(nt + 1) * 512],
                                    start=(ec == 0), stop=(ec == NE - 1))
                    for mi in range(2):
                        m = 2 * mp + mi
                        dest = (QT_s, QT_s, KT_s, KT_s, VT_s, VT_s)[m]
                        bias = bq_s[:, m:m + 1] if m < 4 else 0.0
                        mm = m % 2
                        for nt in range(NJ):
                            nc.scalar.activation(
                                dest[:, mm, nt * 512:(nt + 1) * 512],
                                pss[4 * mi + nt][:],
                                AF.Identity, bias=bias, scale=1.0)


            # V.T -> V natural [t, vcol] via PE transpose (bias folded
            # in later via attention row-sums); own PSUM pool, after ps1
            # is released
            with tc.tile_pool(name="psT", bufs=4, space="PSUM") as psTp:
                for mt in range(NT):
                    for c in range(2):
                        psT = psTp.tile([128, 128], dt, tag="psT",
                                        name=f"psT{mt}_{c}")
                        nc.tensor.transpose(
                            psT[:], VT_s[:, c, mt * 128:(mt + 1) * 128],
                            id_s[:])
                        nc.vector.tensor_copy(
                            V_s[:, mt, 2 * c:2 * c + 2, 0:D],
                            psT[:].rearrange("p (a b) -> p a b", a=2))

            # ------- phase 2: attention + row-split out proj + RS -------
            with tc.tile_pool(name="attn", bufs=2) as attnp, \
                 tc.tile_pool(name="exps", bufs=4) as expsp, \
                 tc.tile_pool(name="out3", bufs=3) as out3, \
                 tc.tile_pool(name="psS", bufs=2, space="PSUM") as psSp, \
                 tc.tile_pool(name="psO", bufs=2, space="PSUM") as psOp, \
                 tc.tile_pool(name="ps3", bufs=2, space="PSUM") as ps3:
                def emit_outproj(jj):
                    last = jj == NJ - 1
                    ytj = out3.tile([128, NE, 512], dt, tag="ytj",
                                    name=f"ytj{jj}")
                    nc.sync.dma_start(
                        out=ytj[:],
                        in_=ytfull[jj].ap().rearrange("(c p) t -> p c t",
                                                      p=128))
                    for mc in range(2):
                        ps = ps3.tile([128, 512], F32, tag="psP",
                                      name=f"psP{jj}_{mc}")
                        for ec in range(NE):
                            nc.tensor.matmul(
                                ps[:],
                                wo_s[:, ec, mc * 128:(mc + 1) * 128],
                                ytj[:, ec, :],
                                start=(ec == 0), stop=(ec == NE - 1))
                        ot = out3.tile([128, 512], dt, tag="ot",
                                       name=f"ot{jj}_{mc}")
                        if last:
                            # the DVE is busy with the last tile's softmax
                            # normalize here; Scalar has finished all exp
                            nc.scalar.add(ot[:], ps[:], bo_s[:, mc:mc + 1])
                        else:
                            nc.vector.tensor_scalar_add(
                                ot[:], ps[:], bo_s[:, mc:mc + 1])
                        nc.sync.dma_start(
                            out=out_shard[jj][mc * 128:(mc + 1) * 128, :],
                            in_=ot[:])

                for j in range(NJ):
                    OTn = attnp.tile([128, 2, 512], dt, tag="otn",
                                     name=f"otn{j}")
                    nkc = 4 * j + 4
                    ex_tiles = [None] * HL

                    def emit_scores(h, j=j, nkc=nkc, ex_tiles=ex_tiles):
                        b64 = 64 * (h % 2)
                        hh = h // 2
                        expS = expsp.tile([128, NT, 512], dt, tag="expS",
                                          name=f"exp{j}_{h}")
                        ex_tiles[h] = expS
                        for kp in range(nkc // 2):
                            ps = psSp.tile([128, 2, 512], F32, tag="psS")
                            offs = []
                            for half in range(2):
                                kc = 2 * kp + half
                                # columns q' < off are fully masked: the
                                # matmul, exp and AV all skip them
                                off = max(0, 128 * kc - 512 * j)
                                offs.append(off)
                                nc.tensor.matmul(
                                    ps[:, half, off:512],
                                    KT_s[b64:b64 + 64, hh,
                                         kc * 128:(kc + 1) * 128],
                                    QT_s[b64:b64 + 64, hh,
                                         j * 512 + off:(j + 1) * 512],
                                    start=True, stop=True)
                                if 128 * kc >= 512 * j:
                                    # diagonal block: additive triangle
                                    nc.vector.tensor_tensor(
                                        out=ps[:, half, off:off + 128],
                                        in0=ps[:, half, off:off + 128],
                                        in1=tri[:], op=OP.add)
                            if offs == [0, 0]:
                                nc.scalar.activation(
                                    expS[:, 2 * kp:2 * kp + 2, :], ps[:],
                                    AF.Exp, scale=SCALE)
                            else:
                                for half in range(2):
                                    kc = 2 * kp + half
                                    nc.scalar.activation(
                                        expS[:, kc, offs[half]:512],
                                        ps[:, half, offs[half]:512],
                                        AF.Exp, scale=SCALE)

                    def emit_av(h, j=j, nkc=nkc, ex_tiles=ex_tiles, OTn=OTn):
                        b64 = 64 * (h % 2)
                        hh = h // 2
                        po = psOp.tile([D + 1, 512], F32, tag="psO")
                        for kc in range(nkc):
                            off = max(0, 128 * kc - 512 * j)
                            nc.tensor.matmul(
                                po[:, off:512], V_s[:, kc, h, :],
                                ex_tiles[h][:, kc, off:512],
                                start=(kc == 0), stop=(kc == nkc - 1))
                        # normalize head h off the PE stream
                        dn = attnp.tile([1, 512], F32, tag="dn")
                        rr = attnp.tile([1, 512], F32, tag="rr")
                        rba = attnp.tile([64, 512], F32, tag="rba")
                        nc.vector.tensor_copy(dn[:], po[D:D + 1, :])
                        nc.vector.reciprocal_approx_fast(rr[:], dn[:])
                        nc.gpsimd.partition_broadcast(rba[:], rr[:])
                        dst = OTn[b64:b64 + 64, hh, :]
                        nc.vector.tensor_tensor(
                            out=dst, in0=po[0:D, :], in1=rba[:], op=OP.mult)
                        # + b_qkv V-slice (attention rows sum to 1)
                        nc.vector.tensor_scalar_add(
                            dst, dst, bq_s[b64:b64 + 64, 4 + hh:5 + hh])
                        if h % 2 == 1:
                            c2 = h // 2
                            nc.sync.dma_start(
                                out=ylocal[j].ap()[c2 * 128:(c2 + 1) * 128, :],
                                in_=OTn[:, c2, :])

                    # head-pipelined schedule: exp(h+1) overlaps AV(h);
                    # out projection of tile j-2 so its gather has ~2
                    # tiles of slack before anything waits on it
                    emit_scores(0)
                    emit_scores(1)
                    if j >= 2:
                        emit_outproj(j - 2)
                    emit_av(0)
                    emit_scores(2)
                    emit_av(1)
                    emit_scores(3)
                    emit_av(2)
                    emit_av(3)
                    # all-gather this tile's y.T within the batch group
                    nc.gpsimd.collective_compute(
                        "AllGather", OP.bypass, replica_groups=GROUPS,
                        ins=[ylocal[j].ap()], outs=[ytfull[j].ap()])
                emit_outproj(NJ - 2)
                emit_outproj(NJ - 1)
    nc.compile()
    return nc


def _get_nc(dt=BF16):
    key = ("nc", dt)
    if key not in _CACHE:
        _CACHE[key] = build_bass(dt)
    return _CACHE[key]


def kernel(x, W_qkv, b_qkv, W_out, b_out, dt=BF16, **run_kwargs):
    import ml_dtypes
    bf16 = ml_dtypes.bfloat16
    x = np.asarray(x, np.float32)
    W_qkv = np.asarray(W_qkv, np.float32)
    b_qkv = np.asarray(b_qkv, np.float32)
    W_out = np.asarray(W_out, np.float32)
    b_out = np.asarray(b_out, np.float32)

    ones2 = np.ones((128, NT * HL), bf16)
    ident = np.eye(128, dtype=bf16)
    in_maps = []
    for c in range(8):
        b, g = divmod(c, 4)
        cols = slice(g * DL, (g + 1) * DL)
        wq = W_qkv[:, 0 * E:1 * E][:, cols]
        wk = W_qkv[:, 1 * E:2 * E][:, cols]
        wv = W_qkv[:, 2 * E:3 * E][:, cols]
        bq = b_qkv[0 * E:1 * E][cols]
        bk = b_qkv[1 * E:2 * E][cols]
        bv = b_qkv[2 * E:3 * E][cols]
        in_maps.append({
            "xT": np.ascontiguousarray(x[b].T).astype(bf16),
            "wqkv": np.concatenate([wq, wk, wv], axis=1).astype(bf16),
            "bqkv": np.concatenate([bq, bk, bv]).reshape(6, 128),
            "wout": np.ascontiguousarray(W_out[:, cols]).astype(bf16),
            "bout": np.ascontiguousarray(b_out[cols]).reshape(2, 128),
            "ones2": ones2,
            "ident": ident,
        })

    res = run_bass_kernel_spmd(_get_nc(dt), in_maps, list(range(8)), **run_kwargs)
    _CACHE["last_results"] = res

    out = np.empty((B, T, E), np.float32)
    for c in range(8):
        b, g = divmod(c, 4)
        shard = res.results[c]["out_shard"]          # [NJ, DL, 512] bf16
        full = shard.transpose(1, 0, 2).reshape(DL, T).astype(np.float32)
        out[b][:, g * DL:(g + 1) * DL] = full.T
    return out


# revision 17
# speedup vs baseline: 1.1798x; 1.0021x over previous
"""Causal self-attention on 8 Trainium2 NeuronCores.

Problem: B=2, T=2048, E=1024, H=16 heads (D=64), fp32.
  qkv = x @ W_qkv + b_qkv ; causal softmax attention ; y @ W_out + b_out

Sharding: core c handles batch b = c//4 and head group g = c%4 (4 heads,
256 of the 1024 hidden dims).  QKV + attention are computed fully
locally per core (tensor-parallel on heads, data-parallel on batch).
The output projection is Megatron ROW-split: each core multiplies its
own y rows [256, q] by W_out[own 256 rows, all 1024 cols], producing a
full-width partial out.T [1024, 512] per q-tile (bf16), and a per-tile
ReduceScatter(add) within the batch group sums the partials; a final
bias+fp32-cast pass lands each core's 256-row shard in the output.
Unlike an AllGather formulation this puts no collective on the critical
path of any matmul; only the last tile's ReduceScatter trails compute.

x / W_qkv / W_out are pre-cast to bf16 on the host (numerically
identical to the on-device cast the kernel would otherwise do, but
halves input DMA bytes and frees the DVE).  PSUM accumulation is fp32
throughout.  Attention uses the transposed-scores layout: S.T[k, q]
tiles so the softmax denominator comes from an appended ones-column in
the V stationary operand and exp() runs on the Scalar engine straight
out of PSUM.  Causal masking is an additive -1e9 on the diagonal
k-chunks.  Per q-tile the heads are software-pipelined
(S0 S1 [outproj j-1] A0 S2 A1 S3 A2 A3) so Scalar-engine exp of head
h+1 overlaps the AV matmuls of head h — exp is the binding engine of
the attention phase — while keeping the PE stream dense enough to hold
the high p-state (the PE ramps 1.2 -> 2.4 GHz only after ~3us of
continuous execution).  The softmax normalize (single-op DVE
reciprocal approx, ~18 bits -> partition broadcast -> multiply) is
emitted per head, off the PE stream.  A tiny warmup AllGather absorbs
cross-core launch skew, triggered only after every input-DMA issue so
it cannot stall the x stream.
"""

import numpy as np

import concourse.bass as bass
import concourse.mybir as mybir
import concourse.tile as tile
from concourse import bacc
from concourse.bass_utils import run_bass_kernel_spmd

F32 = mybir.dt.float32
BF16 = mybir.dt.bfloat16
AF = mybir.ActivationFunctionType
OP = mybir.AluOpType

B, T, E, H = 2, 2048, 1024, 16
D = E // H            # 64 head dim
HL = 4                # heads per core
DL = HL * D           # 256 local hidden dims per core
NE = E // 128         # 8 contraction chunks
NT = T // 128         # 16 t-chunks
NJ = T // 512         # 4 q-tiles
SCALE = 1.0 / float(np.sqrt(D))
NEG = -1.0e9
GROUPS = [[0, 1, 2, 3], [4, 5, 6, 7]]

_CACHE = {}


def build_bass(dt=BF16):
    nc = bacc.Bacc("TRN2", target_bir_lowering=False, debug=False, num_devices=8)

    xT = nc.dram_tensor("xT", [E, T], dt, kind="ExternalInput")
    wqkv = nc.dram_tensor("wqkv", [E, 3 * DL], dt, kind="ExternalInput")
    bqkv = nc.dram_tensor("bqkv", [6, 128], F32, kind="ExternalInput")
    wout = nc.dram_tensor("wout", [E, DL], dt, kind="ExternalInput")
    bout = nc.dram_tensor("bout", [2, 128], F32, kind="ExternalInput")
    ones2 = nc.dram_tensor("ones2", [128, NT * HL], dt, kind="ExternalInput")
    ident = nc.dram_tensor("ident", [128, 128], dt, kind="ExternalInput")
    out_shard = nc.dram_tensor("out_shard", [NJ, DL, 512], dt,
                               kind="ExternalOutput")

    # per-tile y.T staging for the pipelined AllGather.  One tensor PER
    # TILE: dram dependency tracking is per-tensor, so a shared tensor
    # would chain tile j's writes behind the in-flight gather of j-1.
    ylocal = [nc.dram_tensor(f"ylocal{j}", [DL, 512], dt)
              for j in range(NJ)]
    ytfull = [nc.dram_tensor(f"ytfull{j}", [E, 512], dt)
              for j in range(NJ)]
    # last tile: per-head-pair halves so the first gather fires mid-tile
    ylast = [nc.dram_tensor(f"ylast{c}", [128, 512], dt) for c in range(2)]
    ytlast = [nc.dram_tensor(f"ytlast{c}", [512, 512], dt) for c in range(2)]
    warm_in = nc.dram_tensor("warm_in", [1, 512], F32)
    warm_out = nc.dram_tensor("warm_out", [4, 512], F32)

    with tile.TileContext(nc) as tc:
        with tc.tile_pool(name="const", bufs=1) as constp, \
             tc.tile_pool(name="qkvs", bufs=1) as qkvp:
            bq_s = constp.tile([128, 6], F32)
            bo_s = constp.tile([128, 2], F32)
            tri = constp.tile([128, 128], F32)
            warm_s = constp.tile([1, 512], F32)
            id_s = constp.tile([128, 128], dt)

            QT_s = qkvp.tile([128, 2, T], dt)
            KT_s = qkvp.tile([128, 2, T], dt)
            V_s = qkvp.tile([128, NT, HL, D + 1], dt)
            VT_s = qkvp.tile([128, 2, T], dt)
            wo_s = qkvp.tile([128, NE, DL], dt)

            # ---------------- phase 1: QKV projections ----------------
            with tc.tile_pool(name="in1", bufs=1) as in1, \
                 tc.tile_pool(name="ps1", bufs=8, space="PSUM") as ps1:
                xT_r = xT.ap().rearrange("(c p) t -> p c t", p=128)
                wq_r = wqkv.ap().rearrange("(c p) m -> p c m", p=128)
                wo_r = wout.ap().rearrange("(c p) m -> p c m", p=128)

                x_s = in1.tile([128, NE, T], dt, tag="xb")
                w_s = in1.tile([128, NE, 3 * DL], dt, tag="wb")

                # x stream alone on the gpsimd queue; w + small constants
                # on the sync queue
                nc.sync.dma_start(out=w_s[:, 0, :], in_=wq_r[:, 0, :])
                nc.gpsimd.dma_start(out=x_s[:, 0, :], in_=xT_r[:, 0, :])
                nc.sync.dma_start(
                    out=bq_s[:], in_=bqkv.ap().rearrange("m p -> p m"))
                nc.sync.dma_start(
                    out=bo_s[:], in_=bout.ap().rearrange("m p -> p m"))
                nc.sync.dma_start(
                    out=V_s[:, :, :, D],
                    in_=ones2.ap().rearrange("p (a b) -> p a b", a=NT))
                nc.gpsimd.memset(warm_s[:], 0.0)
                nc.sync.dma_start(out=warm_in.ap(), in_=warm_s[:])
                nc.sync.dma_start(out=id_s[:], in_=ident.ap())
                for ec in range(1, NE):
                    nc.sync.dma_start(out=w_s[:, ec, :], in_=wq_r[:, ec, :])
                    if ec < 4:
                        nc.gpsimd.dma_start(out=x_s[:, ec, :],
                                            in_=xT_r[:, ec, :])
                for ec in range(4, NE):
                    nc.sync.dma_start(out=x_s[:, ec, :], in_=xT_r[:, ec, :])
                for ec in range(NE):
                    nc.sync.dma_start(out=wo_s[:, ec, :], in_=wo_r[:, ec, :])
                # warmup collective: rendezvous the group early so launch
                # skew is absorbed here, not in the first ReduceScatter.
                # Triggered after every input DMA issue — the trigger
                # blocks the gpsimd stream until warm_in lands.
                nc.gpsimd.collective_compute(
                    "AllGather", OP.bypass, replica_groups=GROUPS,
                    ins=[warm_in.ap()], outs=[warm_out.ap()])
                nc.gpsimd.memset(tri[:], 0.0)
                nc.gpsimd.affine_select(
                    out=tri[:], in_=tri[:], compare_op=OP.is_ge,
                    fill=NEG, base=0, pattern=[[1, 128]],
                    channel_multiplier=-1)

                # Q.T / K.T : [cols 256, T] each, cols on partitions.
                # Two m-chunks at a time (8 PSUM banks) so the PE has 2x
                # work per x-chunk while the input DMA is still streaming.
                for mp in range(3):
                    pss = [ps1.tile([128, 512], F32, tag="psQ",
                                    name=f"psQ{mp}_{_i}") for _i in range(8)]
                    for ec in range(NE):
                        for mi in range(2):
                            m = 2 * mp + mi
                            for nt in range(NJ):
                                nc.tensor.matmul(
                                    pss[4 * mi + nt][:],
                                    w_s[:, ec, m * 128:(m + 1) * 128],
                                    x_s[:, ec, nt * 512:(nt + 1) * 512],
                                    start=(ec == 0), stop=(ec == NE - 1))
                    for mi in range(2):
                        m = 2 * mp + mi
                        dest = (QT_s, QT_s, KT_s, KT_s, VT_s, VT_s)[m]
                        bias = bq_s[:, m:m + 1] if m < 4 else 0.0
                        mm = m % 2
                        for nt in range(NJ):
                            nc.scalar.activation(
                                dest[:, mm, nt * 512:(nt + 1) * 512],
                                pss[4 * mi + nt][:],
                                AF.Identity, bias=bias, scale=1.0)


            # V.T -> V natural [t, vcol] via PE transpose (bias folded
            # in later via attention row-sums); own PSUM pool, after ps1
            # is released
            with tc.tile_pool(name="psT", bufs=4, space="PSUM") as psTp:
                for mt in range(NT):
                    for c in range(2):
                        psT = psTp.tile([128, 128], dt, tag="psT",
                                        name=f"psT{mt}_{c}")
                        nc.tensor.transpose(
                            psT[:], VT_s[:, c, mt * 128:(mt + 1) * 128],
                            id_s[:])
                        nc.vector.tensor_copy(
                            V_s[:, mt, 2 * c:2 * c + 2, 0:D],
                            psT[:].rearrange("p (a b) -> p a b", a=2))

            # ------- phase 2: attention + row-split out proj + RS -------
            with tc.tile_pool(name="attn", bufs=2) as attnp, \
                 tc.tile_pool(name="exps", bufs=4) as expsp, \
                 tc.tile_pool(name="out3", bufs=3) as out3, \
                 tc.tile_pool(name="psS", bufs=2, space="PSUM") as psSp, \
                 tc.tile_pool(name="psO", bufs=2, space="PSUM") as psOp, \
                 tc.tile_pool(name="ps3", bufs=2, space="PSUM") as ps3:
                def emit_outproj(jj):
                    last = jj == NJ - 1
                    ytj = out3.tile([128, 2, NJ, 512], dt, tag="ytj",
                                    name=f"ytj{jj}")
                    if last:
                        # two half-gathers: rows of ytlast[a] are the four
                        # cores' head-pair-a 128-blocks -> even/odd ec
                        for a in range(2):
                            nc.sync.dma_start(
                                out=ytj[:, a, :, :],
                                in_=ytlast[a].ap().rearrange(
                                    "(g p) t -> p g t", p=128))
                    else:
                        yt4 = ytfull[jj].ap().rearrange(
                            "(g a p) t -> p a g t", a=2, p=128)
                        for a in range(2):
                            nc.sync.dma_start(out=ytj[:, a, :, :],
                                              in_=yt4[:, a, :, :])
                    # ec chunk e lives at ytj[:, e%2, e//2, :]; for the last
                    # tile the a=0 half arrives one gather earlier, so run
                    # the even-ec accumulation first
                    ec_order = ([0, 2, 4, 6, 1, 3, 5, 7] if last
                                else list(range(NE)))
                    for mc in range(2):
                        ps = ps3.tile([128, 512], F32, tag="psP",
                                      name=f"psP{jj}_{mc}")
                        for ei, ec in enumerate(ec_order):
                            nc.tensor.matmul(
                                ps[:],
                                wo_s[:, ec, mc * 128:(mc + 1) * 128],
                                ytj[:, ec % 2, ec // 2, :],
                                start=(ei == 0), stop=(ei == NE - 1))
                        ot = out3.tile([128, 512], dt, tag="ot",
                                       name=f"ot{jj}_{mc}")
                        if last:
                            # the DVE is busy with the last tile's softmax
                            # normalize here; Scalar has finished all exp
                            nc.scalar.add(ot[:], ps[:], bo_s[:, mc:mc + 1])
                        else:
                            nc.vector.tensor_scalar_add(
                                ot[:], ps[:], bo_s[:, mc:mc + 1])
                        nc.sync.dma_start(
                            out=out_shard[jj][mc * 128:(mc + 1) * 128, :],
                            in_=ot[:])

                for j in range(NJ):
                    OTn = attnp.tile([128, 2, 512], dt, tag="otn",
                                     name=f"otn{j}")
                    nkc = 4 * j + 4
                    ex_tiles = [None] * HL

                    def emit_scores(h, j=j, nkc=nkc, ex_tiles=ex_tiles):
                        b64 = 64 * (h % 2)
                        hh = h // 2
                        expS = expsp.tile([128, NT, 512], dt, tag="expS",
                                          name=f"exp{j}_{h}")
                        ex_tiles[h] = expS
                        for kp in range(nkc // 2):
                            ps = psSp.tile([128, 2, 512], F32, tag="psS")
                            offs = []
                            for half in range(2):
                                kc = 2 * kp + half
                                # columns q' < off are fully masked: the
                                # matmul, exp and AV all skip them
                                off = max(0, 128 * kc - 512 * j)
                                offs.append(off)
                                nc.tensor.matmul(
                                    ps[:, half, off:512],
                                    KT_s[b64:b64 + 64, hh,
                                         kc * 128:(kc + 1) * 128],
                                    QT_s[b64:b64 + 64, hh,
                                         j * 512 + off:(j + 1) * 512],
                                    start=True, stop=True)
                                if 128 * kc >= 512 * j:
                                    # diagonal block: additive triangle
                                    nc.vector.tensor_tensor(
                                        out=ps[:, half, off:off + 128],
                                        in0=ps[:, half, off:off + 128],
                                        in1=tri[:], op=OP.add)
                            if offs == [0, 0]:
                                nc.scalar.activation(
                                    expS[:, 2 * kp:2 * kp + 2, :], ps[:],
                                    AF.Exp, scale=SCALE)
                            else:
                                for half in range(2):
                                    kc = 2 * kp + half
                                    nc.scalar.activation(
                                        expS[:, kc, offs[half]:512],
                                        ps[:, half, offs[half]:512],
                                        AF.Exp, scale=SCALE)

                    def emit_av(h, j=j, nkc=nkc, ex_tiles=ex_tiles, OTn=OTn):
                        b64 = 64 * (h % 2)
                        hh = h // 2
                        po = psOp.tile([D + 1, 512], F32, tag="psO")
                        for kc in range(nkc):
                            off = max(0, 128 * kc - 512 * j)
                            nc.tensor.matmul(
                                po[:, off:512], V_s[:, kc, h, :],
                                ex_tiles[h][:, kc, off:512],
                                start=(kc == 0), stop=(kc == nkc - 1))
                        # normalize head h off the PE stream
                        dn = attnp.tile([1, 512], F32, tag="dn")
                        rr = attnp.tile([1, 512], F32, tag="rr")
                        rba = attnp.tile([64, 512], F32, tag="rba")
                        nc.vector.tensor_copy(dn[:], po[D:D + 1, :])
                        nc.vector.reciprocal_approx_fast(rr[:], dn[:])
                        nc.gpsimd.partition_broadcast(rba[:], rr[:])
                        dst = OTn[b64:b64 + 64, hh, :]
                        nc.vector.tensor_tensor(
                            out=dst, in0=po[0:D, :], in1=rba[:], op=OP.mult)
                        # + b_qkv V-slice (attention rows sum to 1)
                        nc.vector.tensor_scalar_add(
                            dst, dst, bq_s[b64:b64 + 64, 4 + hh:5 + hh])
                        if h % 2 == 1:
                            c2 = h // 2
                            if j == NJ - 1:
                                nc.sync.dma_start(out=ylast[c2].ap(),
                                                  in_=OTn[:, c2, :])
                                nc.gpsimd.collective_compute(
                                    "AllGather", OP.bypass,
                                    replica_groups=GROUPS,
                                    ins=[ylast[c2].ap()],
                                    outs=[ytlast[c2].ap()])
                            else:
                                nc.sync.dma_start(
                                    out=ylocal[j].ap()[c2 * 128:(c2 + 1) * 128, :],
                                    in_=OTn[:, c2, :])

                    # head-pipelined schedule: exp(h+1) overlaps AV(h);
                    # out projection of tile j-2 so its gather has ~2
                    # tiles of slack before anything waits on it
                    emit_scores(0)
                    emit_scores(1)
                    if j >= 2:
                        emit_outproj(j - 2)
                    emit_av(0)
                    emit_scores(2)
                    emit_av(1)
                    emit_scores(3)
                    emit_av(2)
                    emit_av(3)
                    if j < NJ - 1:
                        # all-gather this tile's y.T within the batch group
                        nc.gpsimd.collective_compute(
                            "AllGather", OP.bypass, replica_groups=GROUPS,
                            ins=[ylocal[j].ap()], outs=[ytfull[j].ap()])
                emit_outproj(NJ - 2)
                emit_outproj(NJ - 1)
    nc.compile()
    return nc


def _get_nc(dt=BF16):
    key = ("nc", dt)
    if key not in _CACHE:
        _CACHE[key] = build_bass(dt)
    return _CACHE[key]


def kernel(x, W_qkv, b_qkv, W_out, b_out, dt=BF16, **run_kwargs):
    import ml_dtypes
    bf16 = ml_dtypes.bfloat16
    x = np.asarray(x, np.float32)
    W_qkv = np.asarray(W_qkv, np.float32)
    b_qkv = np.asarray(b_qkv, np.float32)
    W_out = np.asarray(W_out, np.float32)
    b_out = np.asarray(b_out, np.float32)

    ones2 = np.ones((128, NT * HL), bf16)
    ident = np.eye(128, dtype=bf16)
    in_maps = []
    for c in range(8):
        b, g = divmod(c, 4)
        cols = slice(g * DL, (g + 1) * DL)
        wq = W_qkv[:, 0 * E:1 * E][:, cols]
        wk = W_qkv[:, 1 * E:2 * E][:, cols]
        wv = W_qkv[:, 2 * E:3 * E][:, cols]
        bq = b_qkv[0 * E:1 * E][cols]
        bk = b_qkv[1 * E:2 * E][cols]
        bv = b_qkv[2 * E:3 * E][cols]
        in_maps.append({
            "xT": np.ascontiguousarray(x[b].T).astype(bf16),
            "wqkv": np.concatenate([wq, wk, wv], axis=1).astype(bf16),
            "bqkv": np.concatenate([bq, bk, bv]).reshape(6, 128),
            "wout": np.ascontiguousarray(W_out[:, cols]).astype(bf16),
            "bout": np.ascontiguousarray(b_out[cols]).reshape(2, 128),
            "ones2": ones2,
            "ident": ident,
        })

    res = run_bass_kernel_spmd(_get_nc(dt), in_maps, list(range(8)), **run_kwargs)
    _CACHE["last_results"] = res

    out = np.empty((B, T, E), np.float32)
    for c in range(8):
        b, g = divmod(c, 4)
        shard = res.results[c]["out_shard"]          # [NJ, DL, 512] bf16
        full = shard.transpose(1, 0, 2).reshape(DL, T).astype(np.float32)
        out[b][:, g * DL:(g + 1) * DL] = full.T
    return out
